# revision 1
# baseline (speedup 1.0000x reference)
"""H2GCN forward pass distributed over 8 TRN2 NeuronCores.

Sharding: nodes row-sharded across cores; edges partitioned by src owner so
the segment_sum is core-local; r_prev all-gathered between layers; weights
replicated.

Device algorithm per SpMM (D^-1 A @ r):
  - edges sorted by src, grouped into 128-node windows; each window's slot
    list padded to a uniform SW (cross-core max) so the SPMD program is
    identical on every core. Pad slots have dst=OOB (gather skipped via
    bounds_check) and weight 0.
  - gather r[dst] rows into SBUF via indirect DMA in [128, GC] batches
  - msgw = msg * w_slot (DVE)
  - per window: S[p, j] = (node_local[p] == j) built in one batched DVE
    is_equal; n_T[64,128] accumulated in PSUM via matmul(lhsT=msgw_tile,
    rhs=S_tile) over the window's tiles.
Dense matmuls keep a transposed layout (features on partitions) so biases are
per-partition ACT adds and no on-device transposes of x are needed (x is
pre-transposed on the host as part of sharding).
"""

import sys

sys.path.insert(0, "/opt/trn_rl_repo")

import numpy as np

import concourse.bacc as bacc
import concourse.bass as bass
import concourse.mybir as mybir
import concourse.tile as tile
from concourse.bass_utils import run_bass_kernel_spmd
from concourse.masks import make_identity

F32 = mybir.dt.float32
I32 = mybir.dt.int32
AF = mybir.ActivationFunctionType
OOB = 1 << 20  # pad-slot dst marker; > bounds_check so the gather skips it
GC = 64  # gather batch width (tiles per indirect DMA call)


def _seg_ranges(counts):
    counts = np.asarray(counts, dtype=np.int64)
    tot = int(counts.sum())
    if tot == 0:
        return np.zeros(0, np.int64)
    starts = np.concatenate([[0], np.cumsum(counts)[:-1]])
    return np.arange(tot) - np.repeat(starts, counts)


def _prep_set(src, dst, n_nodes, ncores):
    """Per-core slot arrays for one edge set: idx [128,T], ws [128,T], nl [128,T]."""
    npc = n_nodes // ncores
    nw = (npc + 127) // 128
    deg = np.bincount(src, minlength=n_nodes)
    wglob = (1.0 / np.maximum(deg, 1.0)).astype(np.float32)

    per_core = []
    sw_req = 0
    for c in range(ncores):
        lo = c * npc
        m = (src >= lo) & (src < lo + npc)
        sl = (src[m] - lo).astype(np.int64)
        order = np.argsort(sl, kind="stable")
        sl = sl[order]
        d = dst[m].astype(np.int64)[order]
        wv = wglob[src[m]][order]
        cnt = np.bincount(sl, minlength=npc).astype(np.int64)
        wid_n = np.arange(npc) // 128
        wslots = np.bincount(wid_n, weights=cnt, minlength=nw).astype(np.int64)
        sw_req = max(sw_req, int(wslots.max()))
        per_core.append((sl, d, wv, cnt))

    SW = max(128, ((sw_req + 127) // 128) * 128)
    T = nw * SW // 128

    outs = []
    for c in range(ncores):
        sl, d, wv, cnt = per_core[c]
        c_ex = np.concatenate([[0], np.cumsum(cnt)])[:-1]
        wid_n = np.arange(npc) // 128
        first = wid_n * 128
        node_off = c_ex - c_ex[first]  # node's slot offset within its window
        gpos = wid_n * SW + node_off  # node's first slot in padded array
        rank = np.arange(len(sl)) - c_ex[sl]
        pos = gpos[sl] + rank
        total = nw * SW
        idx = np.full(total, OOB, np.int32)
        ws = np.zeros(total, np.float32)
        nl = np.zeros(total, np.float32)
        idx[pos] = d
        ws[pos] = wv
        nl[pos] = (sl % 128).astype(np.float32)
        outs.append(
            dict(
                idx=np.ascontiguousarray(idx.reshape(T, 128).T),
                ws=np.ascontiguousarray(ws.reshape(T, 128).T),
                nl=np.ascontiguousarray(nl.reshape(T, 128).T),
            )
        )
    return outs, SW, T


def build_program(cfg):
    n_nodes = cfg["n_nodes"]
    npc = cfg["npc"]
    nw = cfg["nw"]
    ncores = cfg["ncores"]
    ipad = cfg["ipad"]  # padded input dim (mult of 128)
    ncls = cfg["ncls"]
    T = cfg["T"]  # tiles per set: [T0, T1]
    tpw = cfg["tpw"]  # tiles per window per set
    KT = ipad // 128
    H = 64

    nc = bacc.Bacc(
        "TRN2",
        target_bir_lowering=False,
        debug=False,
        enable_asserts=False,
        num_devices=ncores,
    )

    # --- DRAM I/O ---
    xT = nc.dram_tensor("xT", [ipad, npc], F32, kind="ExternalInput")
    wemb = nc.dram_tensor("wemb", [ipad, H], F32, kind="ExternalInput")
    bemb = nc.dram_tensor("bemb", [H, 1], F32, kind="ExternalInput")
    wl = [
        nc.dram_tensor(f"w{i}", [4 * H, H], F32, kind="ExternalInput") for i in (0, 1)
    ]
    bl = [nc.dram_tensor(f"b{i}", [H, 1], F32, kind="ExternalInput") for i in (0, 1)]
    wc = nc.dram_tensor("wc", [256, ncls], F32, kind="ExternalInput")  # zero-padded
    bc = nc.dram_tensor("bc", [ncls, 1], F32, kind="ExternalInput")
    tpwmax = max(tpw)
    iota = nc.dram_tensor("iota", [128, tpwmax, 128], F32, kind="ExternalInput")
    idx_d = [
        nc.dram_tensor(f"idx{s}", [128, T[s]], I32, kind="ExternalInput") for s in (0, 1)
    ]
    ws_d = [
        nc.dram_tensor(f"ws{s}", [128, T[s]], F32, kind="ExternalInput") for s in (0, 1)
    ]
    nl_d = [
        nc.dram_tensor(f"nl{s}", [128, T[s]], F32, kind="ExternalInput") for s in (0, 1)
    ]
    out_d = nc.dram_tensor("logitsT", [ncls, npc], F32, kind="ExternalOutput")

    # internal DRAM
    r_loc = [nc.dram_tensor(f"r{k}_loc", [npc, H], F32) for k in (0, 1)]
    r_tab = [
        nc.dram_tensor(f"r{k}_tab", [n_nodes, H], F32, addr_space="Shared")
        for k in (0, 1)
    ]
    rT = [nc.dram_tensor(f"rT{k}", [H, npc], F32) for k in (0, 1, 2)]

    groups = [list(range(ncores))]

    with tile.TileContext(nc) as tc:
        with (
            tc.tile_pool(name="const", bufs=1) as cp,
            tc.tile_pool(name="io", bufs=2) as iop,
            tc.tile_pool(name="msg", bufs=2) as mp,
            tc.tile_pool(name="meta", bufs=2) as mep,
            tc.tile_pool(name="s2", bufs=2) as s2p,
            tc.tile_pool(name="kt", bufs=3) as ktp,
            tc.tile_pool(name="st", bufs=3) as stp,
        ):
            # constants / weights into SBUF
            ident = cp.tile([128, 128], F32, tag="ident")
            make_identity(nc, ident[:])
            iota_sb = cp.tile([128, tpwmax, 128], F32, tag="iota")
            nc.sync.dma_start(iota_sb[:], iota[:])
            wemb_sb = cp.tile([128, KT, H], F32, tag="wemb")
            nc.sync.dma_start(
                wemb_sb[:], wemb.ap().rearrange("(k p) h -> p k h", p=128)
            )
            wl_sb = []
            for i in (0, 1):
                t = cp.tile([128, 2, H], F32, tag=f"wl{i}")
                nc.sync.dma_start(t[:], wl[i].ap().rearrange("(k p) h -> p k h", p=128))
                wl_sb.append(t)
            wc_sb = cp.tile([128, 2, ncls], F32, tag="wc")
            nc.sync.dma_start(wc_sb[:], wc.ap().rearrange("(k p) h -> p k h", p=128))
            bemb_sb = cp.tile([H, 1], F32, tag="bemb")
            nc.sync.dma_start(bemb_sb[:], bemb[:])
            bl_sb = []
            for i in (0, 1):
                t = cp.tile([H, 1], F32, tag=f"bl{i}")
                nc.sync.dma_start(t[:], bl[i][:])
                bl_sb.append(t)
            bc_sb = cp.tile([ncls, 1], F32, tag="bc")
            nc.sync.dma_start(bc_sb[:], bc[:])
            nl_sb = []
            for s in (0, 1):
                t = cp.tile([128, T[s]], F32, tag=f"nl{s}")
                nc.sync.dma_start(t[:], nl_d[s][:])
                nl_sb.append(t)

            # ---------------- Phase A: r0 = relu(x @ Wemb + b) ----------------
            with tc.tile_pool(name="psA", bufs=2, space="PSUM") as psA:
                for w in range(nw):
                    nodes = min(128, npc - w * 128)
                    xw = iop.tile([128, KT, 128], F32, tag="xw")
                    nc.sync.dma_start(
                        xw[:, :, :nodes],
                        xT.ap().rearrange("(k p) n -> p k n", p=128)[
                            :, :, w * 128 : w * 128 + nodes
                        ],
                    )
                    ps = psA.tile([H, 128], F32, tag="e")
                    for k in range(KT):
                        nc.tensor.matmul(
                            ps[:, :nodes],
                            wemb_sb[:, k, :],
                            xw[:, k, :nodes],
                            start=(k == 0),
                            stop=(k == KT - 1),
                        )
                    r0T_st = stp.tile([H, 128], F32, tag="rkT")
                    nc.scalar.activation(
                        r0T_st[:, :nodes], ps[:, :nodes], AF.Relu, bias=bemb_sb[:, :1]
                    )
                    nc.sync.dma_start(
                        rT[0].ap()[:, w * 128 : w * 128 + nodes], r0T_st[:, :nodes]
                    )
                    pst = psA.tile([128, H], F32, tag="tr")
                    nc.tensor.transpose(
                        pst[:nodes, :], r0T_st[:, :nodes], ident[:H, :H]
                    )
                    r0_st = stp.tile([128, H], F32, tag="rrow")
                    nc.scalar.activation(r0_st[:nodes, :], pst[:nodes, :], AF.Copy)
                    nc.sync.dma_start(
                        r_loc[0].ap()[w * 128 : w * 128 + nodes, :], r0_st[:nodes, :]
                    )

            nc.gpsimd.collective_compute(
                "AllGather",
                mybir.AluOpType.bypass,
                replica_groups=groups,
                ins=[r_loc[0].ap().opt()],
                outs=[r_tab[0].ap().opt()],
            )

            # ---------------- Layers ----------------
            for li in (0, 1):
                table = r_tab[li]
                rT_prev = rT[li]
                with tc.tile_pool(name=f"psB{li}", bufs=1, space="PSUM") as psB:
                    emitted_g = [0, 0]
                    msg_tiles = [[], []]

                    def emit_gather(s, g):
                        lo = g * GC
                        hi = min(lo + GC, T[s])
                        n = hi - lo
                        it = mep.tile([128, GC], I32, tag=f"idx{s}")
                        nc.sync.dma_start(it[:, :n], idx_d[s].ap()[:, lo:hi])
                        wt = mep.tile([128, GC], F32, tag=f"ws{s}")
                        nc.sync.dma_start(wt[:, :n], ws_d[s].ap()[:, lo:hi])
                        mt = mp.tile([128, GC, 64], F32, tag=f"msg{s}")
                        if g < 2:
                            nc.gpsimd.memset(mt[:], 0.0)
                        for tt in range(n):
                            nc.gpsimd.indirect_dma_start(
                                out=mt[:, tt, :],
                                out_offset=None,
                                in_=table.ap(),
                                in_offset=bass.IndirectOffsetOnAxis(
                                    ap=it[:, tt : tt + 1], axis=0
                                ),
                                bounds_check=n_nodes - 1,
                                oob_is_err=False,
                            )
                        nc.vector.tensor_tensor(
                            out=mt[:, :n, :],
                            in0=mt[:, :n, :],
                            in1=wt[:, :n, None].to_broadcast([128, n, 64]),
                            op=mybir.AluOpType.mult,
                        )
                        msg_tiles[s].append(mt)

                    for w in range(nw):
                        nodes = min(128, npc - w * 128)
                        for s in (0, 1):
                            need_g = -(-((w + 1) * tpw[s]) // GC)
                            need_g = min(need_g, -(-T[s] // GC))
                            while emitted_g[s] < need_g:
                                emit_gather(s, emitted_g[s])
                                emitted_g[s] += 1
                        ps2 = []
                        for s in (0, 1):
                            S_w = s2p.tile([128, tpw[s], 128], F32, tag=f"S{s}")
                            base = w * tpw[s]
                            nc.vector.tensor_tensor(
                                out=S_w[:],
                                in0=nl_sb[s][:, base : base + tpw[s], None].to_broadcast(
                                    [128, tpw[s], 128]
                                ),
                                in1=iota_sb[:, : tpw[s], :],
                                op=mybir.AluOpType.is_equal,
                            )
                            ps = psB.tile([H, 128], F32, tag=f"n{s}")
                            for j in range(tpw[s]):
                                t = base + j
                                nc.tensor.matmul(
                                    ps[:, :nodes],
                                    msg_tiles[s][t // GC][:, t % GC, :],
                                    S_w[:, j, :nodes],
                                    start=(j == 0),
                                    stop=(j == tpw[s] - 1),
                                )
                            ps2.append(ps)
                        # concat k-tiles: [r_prevT; n_sT]
                        kts = []
                        for s in (0, 1):
                            kt = ktp.tile([128, 128], F32, tag=f"kt{s}")
                            nc.sync.dma_start(
                                kt[:H, :nodes],
                                rT_prev.ap()[:, w * 128 : w * 128 + nodes],
                            )
                            nc.scalar.activation(
                                kt[H:, :nodes], ps2[s][:, :nodes], AF.Copy
                            )
                            kts.append(kt)
                        ps3 = psB.tile([H, 128], F32, tag="r")
                        for s in (0, 1):
                            nc.tensor.matmul(
                                ps3[:, :nodes],
                                wl_sb[li][:, s, :],
                                kts[s][:, :nodes],
                                start=(s == 0),
                                stop=(s == 1),
                            )
                        rkT_st = stp.tile([H, 128], F32, tag="rkT")
                        nc.scalar.activation(
                            rkT_st[:, :nodes],
                            ps3[:, :nodes],
                            AF.Relu,
                            bias=bl_sb[li][:, :1],
                        )
                        nc.sync.dma_start(
                            rT[li + 1].ap()[:, w * 128 : w * 128 + nodes],
                            rkT_st[:, :nodes],
                        )
                        if li == 0:
                            pst = psB.tile([128, H], F32, tag="tr")
                            nc.tensor.transpose(
                                pst[:nodes, :], rkT_st[:, :nodes], ident[:H, :H]
                            )
                            rk_st = stp.tile([128, H], F32, tag="rrow")
                            nc.scalar.activation(
                                rk_st[:nodes, :], pst[:nodes, :], AF.Copy
                            )
                            nc.sync.dma_start(
                                r_loc[1].ap()[w * 128 : w * 128 + nodes, :],
                                rk_st[:nodes, :],
                            )
                if li == 0:
                    nc.gpsimd.collective_compute(
                        "AllGather",
                        mybir.AluOpType.bypass,
                        replica_groups=groups,
                        ins=[r_loc[1].ap().opt()],
                        outs=[r_tab[1].ap().opt()],
                    )

            # ---------------- Logits ----------------
            with tc.tile_pool(name="psC", bufs=2, space="PSUM") as psC:
                for w in range(nw):
                    nodes = min(128, npc - w * 128)
                    kt1 = ktp.tile([128, 128], F32, tag="lkt1")
                    nc.sync.dma_start(
                        kt1[:H, :nodes], rT[0].ap()[:, w * 128 : w * 128 + nodes]
                    )
                    nc.sync.dma_start(
                        kt1[H:, :nodes], rT[1].ap()[:, w * 128 : w * 128 + nodes]
                    )
                    kt2 = ktp.tile([H, 128], F32, tag="lkt2")
                    nc.sync.dma_start(
                        kt2[:, :nodes], rT[2].ap()[:, w * 128 : w * 128 + nodes]
                    )
                    ps5 = psC.tile([ncls, 128], F32, tag="lg")
                    nc.tensor.matmul(
                        ps5[:, :nodes], wc_sb[:, 0, :], kt1[:, :nodes],
                        start=True, stop=False,
                    )
                    nc.tensor.matmul(
                        ps5[:, :nodes], wc_sb[:H, 1, :], kt2[:, :nodes],
                        start=False, stop=True,
                    )
                    lg_st = stp.tile([ncls, 128], F32, tag="lg")
                    nc.scalar.activation(
                        lg_st[:, :nodes], ps5[:, :nodes], AF.Identity, bias=bc_sb[:, :1]
                    )
                    nc.sync.dma_start(
                        out_d.ap()[:, w * 128 : w * 128 + nodes], lg_st[:, :nodes]
                    )

    nc.compile()
    return nc


def prepare(x, edge_index_1, edge_index_2, W_embed, b_embed, W0, b0, W1, b1, Wc, bc,
            ncores=8):
    x = np.asarray(x, np.float32)
    n_nodes, in_dim = x.shape
    npc = n_nodes // ncores
    nw = (npc + 127) // 128
    ipad = ((in_dim + 127) // 128) * 128
    ncls = np.asarray(Wc).shape[1]

    e1 = np.asarray(edge_index_1)
    e2 = np.asarray(edge_index_2)
    set0, SW0, T0 = _prep_set(e1[0], e1[1], n_nodes, ncores)
    set1, SW1, T1 = _prep_set(e2[0], e2[1], n_nodes, ncores)

    wemb_p = np.zeros((ipad, 64), np.float32)
    wemb_p[:in_dim] = np.asarray(W_embed, np.float32)
    wc_p = np.zeros((256, ncls), np.float32)
    wc_p[: np.asarray(Wc).shape[0]] = np.asarray(Wc, np.float32)
    tpwmax = max(SW0 // 128, SW1 // 128)
    iota = np.broadcast_to(np.arange(128, dtype=np.float32), (128, tpwmax, 128)).copy()

    shared = {
        "wemb": wemb_p,
        "bemb": np.asarray(b_embed, np.float32).reshape(64, 1),
        "w0": np.asarray(W0, np.float32),
        "b0": np.asarray(b0, np.float32).reshape(64, 1),
        "w1": np.asarray(W1, np.float32),
        "b1": np.asarray(b1, np.float32).reshape(64, 1),
        "wc": wc_p,
        "bc": np.asarray(bc, np.float32).reshape(ncls, 1),
        "iota": iota,
    }
    in_maps = []
    for c in range(ncores):
        xTc = np.zeros((ipad, npc), np.float32)
        xTc[:in_dim] = x[c * npc : (c + 1) * npc].T
        m = dict(shared)
        m["xT"] = np.ascontiguousarray(xTc)
        for s, st in ((0, set0), (1, set1)):
            m[f"idx{s}"] = st[c]["idx"]
            m[f"ws{s}"] = st[c]["ws"]
            m[f"nl{s}"] = st[c]["nl"]
        in_maps.append(m)

    cfg = dict(
        n_nodes=n_nodes, npc=npc, nw=nw, ncores=ncores, ipad=ipad, ncls=ncls,
        T=[T0, T1], tpw=[SW0 // 128, SW1 // 128],
    )
    return cfg, in_maps


_CACHE = {}


def kernel(**inputs):
    ncores = 8
    cfg, in_maps = prepare(**inputs, ncores=ncores)
    key = str(sorted(cfg.items()))
    if key not in _CACHE:
        _CACHE[key] = build_program(cfg)
    nc = _CACHE[key]
    res = run_bass_kernel_spmd(nc, in_maps, core_ids=list(range(ncores)))
    ncls, npc = cfg["ncls"], cfg["npc"]
    out = np.empty((cfg["n_nodes"], ncls), np.float32)
    for c in range(ncores):
        out[c * npc : (c + 1) * npc] = res.results[c]["logitsT"].T
    return out



# revision 2
# speedup vs baseline: 1.0030x; 1.0030x over previous
"""H2GCN forward on 8 TRN2 NeuronCores — v3.

Gather via batched dma_gather (mlp-library ucode, single_packet=False):
~5K indices per call instead of 128 per indirect-DMA call (the v1
bottleneck: ~1.3us SWDGE fixed cost per call, 5.3K calls = 6.8ms on the
GpSimd engine). int16 index constraint handled by slicing the gathered
table into 4 quarters (2 core-blocks each, 25088 rows < 32767) and
sorting each call's slots by (window, quarter). Segment-sum via one-hot
S matmuls (S built on DVE from slot labels; handles ragged slots and
masks pads with label -1). Dense layers in bf16 row-major with
bias-row/ones-row folding; f32 table (dma_gather needs 256B payloads).
"""

import sys

sys.path.insert(0, "/opt/trn_rl_repo")

import numpy as np
from ml_dtypes import bfloat16

import concourse.bacc as bacc
import concourse.bass as bass
import concourse.mybir as mybir
import concourse.tile as tile
from concourse import library_config
from concourse.bass_utils import run_bass_kernel_spmd
from concourse.masks import make_identity

F32 = mybir.dt.float32
BF16 = mybir.dt.bfloat16
I16 = mybir.dt.int16
AF = mybir.ActivationFunctionType
EQ = mybir.AluOpType.is_equal

NCORES = 8
H = 64
W_B = 14  # windows per gather block
NQ = 4  # table quarters


def build_program(cfg):
    NW, KT, NCLS, NP, NTAB = (cfg[k] for k in ("NW", "KT", "NCLS", "NP", "NTAB"))
    nidx = cfg["nidx"]  # [s][b][q] -> padded index count per call
    sched = cfg["sched"]  # per (b): list of (s, q, tile, w_in_b, col, first, last)
    npieces = cfg["npieces"]  # [s] total pieces (nl columns) per set
    NB = NW // W_B
    QROWS = NTAB // NQ

    nc = bacc.Bacc(
        "TRN2",
        target_bir_lowering=False,
        debug=False,
        enable_asserts=False,
        num_devices=NCORES,
    )

    xpre = nc.dram_tensor("xpre", [128, NW * KT * 128], BF16, kind="ExternalInput")
    wembP = nc.dram_tensor("wembP", [128, KT * H], BF16, kind="ExternalInput")
    brow_emb = nc.dram_tensor("brow_emb", [1, H], BF16, kind="ExternalInput")
    wacc = [
        nc.dram_tensor(f"wacc{i}", [H + 1, H], BF16, kind="ExternalInput")
        for i in (0, 1)
    ]
    wb = [nc.dram_tensor(f"wb{i}", [H, H], BF16, kind="ExternalInput") for i in (0, 1)]
    wd = [nc.dram_tensor(f"wd{i}", [H, H], BF16, kind="ExternalInput") for i in (0, 1)]
    wc0 = nc.dram_tensor("wc0", [H + 1, NCLS], BF16, kind="ExternalInput")
    wc1 = nc.dram_tensor("wc1", [H, NCLS], BF16, kind="ExternalInput")
    wc2 = nc.dram_tensor("wc2", [H, NCLS], BF16, kind="ExternalInput")
    XW = [sum(nidx[s][b][q] for b in range(NB) for q in range(NQ)) // 16
          for s in (0, 1)]
    idx_d = [
        nc.dram_tensor(f"idx{s}", [128, XW[s]], I16, kind="ExternalInput")
        for s in (0, 1)
    ]
    nl_d = [
        nc.dram_tensor(f"nl{s}", [128, npieces[s]], BF16, kind="ExternalInput")
        for s in (0, 1)
    ]
    invd_d = [
        nc.dram_tensor(f"invd{s}", [128, NW], F32, kind="ExternalInput")
        for s in (0, 1)
    ]
    iota_d = nc.dram_tensor("iota", [128, 128], BF16, kind="ExternalInput")
    onesrow = nc.dram_tensor("onesrow", [1, NP], BF16, kind="ExternalInput")
    out_d = nc.dram_tensor("logits", [NP, NCLS], F32, kind="ExternalOutput")

    rloc = [nc.dram_tensor(f"rloc{k}", [NP, H], F32) for k in (0, 1)]
    tab = [
        nc.dram_tensor(f"tab{k}", [NTAB, H], F32, addr_space="Shared")
        for k in (0, 1)
    ]
    replica = [list(range(NCORES))]

    with tile.TileContext(nc) as tc:
        with (
            tc.tile_pool(name="const", bufs=1) as cp,
            tc.tile_pool(name="io", bufs=2) as iop,
            tc.tile_pool(name="msg", bufs=2) as mp,
            tc.tile_pool(name="msgb", bufs=1) as mpb,
            tc.tile_pool(name="meta", bufs=2) as mep,
            tc.tile_pool(name="st", bufs=3) as stp,
        ):
            ident = cp.tile([128, 128], BF16, tag="ident")
            make_identity(nc, ident[:])
            nc.gpsimd.load_library(library_config.mlp)
            iota_sb = cp.tile([128, 128], BF16, tag="iota")
            nc.sync.dma_start(iota_sb[:], iota_d[:])
            wembP_sb = cp.tile([128, KT, H], BF16, tag="wembP")
            nc.sync.dma_start(
                wembP_sb[:], wembP.ap().rearrange("p (k h) -> p k h", k=KT)
            )
            bemb_sb = cp.tile([1, H], BF16, tag="bemb")
            nc.sync.dma_start(bemb_sb[:], brow_emb[:])
            ones1_sb = cp.tile([1, 128], BF16, tag="ones1")
            nc.sync.dma_start(ones1_sb[:], onesrow.ap()[:, :128])
            wacc_sb, wb_sb, wd_sb = [], [], []
            for i in (0, 1):
                t = cp.tile([H + 1, H], BF16, tag=f"wacc{i}")
                nc.sync.dma_start(t[:], wacc[i][:])
                wacc_sb.append(t)
                t = cp.tile([H, H], BF16, tag=f"wb{i}")
                nc.sync.dma_start(t[:], wb[i][:])
                wb_sb.append(t)
                t = cp.tile([H, H], BF16, tag=f"wd{i}")
                nc.sync.dma_start(t[:], wd[i][:])
                wd_sb.append(t)
            wc0_sb = cp.tile([H + 1, NCLS], BF16, tag="wc0")
            nc.sync.dma_start(wc0_sb[:], wc0[:])
            wc1_sb = cp.tile([H, NCLS], BF16, tag="wc1")
            nc.sync.dma_start(wc1_sb[:], wc1[:])
            wc2_sb = cp.tile([H, NCLS], BF16, tag="wc2")
            nc.sync.dma_start(wc2_sb[:], wc2[:])
            invd_sb = []
            for s in (0, 1):
                t = cp.tile([128, NW], F32, tag=f"invd{s}")
                nc.sync.dma_start(t[:], invd_d[s][:])
                invd_sb.append(t)
            nl_sb = []
            for s in (0, 1):
                t = cp.tile([128, npieces[s]], BF16, tag=f"nl{s}")
                nc.sync.dma_start(t[:], nl_d[s][:])
                nl_sb.append(t)

            rT0 = cp.tile([H + 1, NP], BF16, tag="rT0")
            rT1 = cp.tile([H + 1, NP], BF16, tag="rT1")
            rT2 = cp.tile([H, NP], BF16, tag="rT2")
            rT = [rT0, rT1, rT2]
            nc.sync.dma_start(rT0[H : H + 1, :], onesrow[:])
            nc.sync.dma_start(rT1[H : H + 1, :], onesrow[:])

            # per-call idx column offsets (wrapped layout, 16 slots/col)
            xoff = [[[0] * NQ for _ in range(NB)] for _ in (0, 1)]
            for s in (0, 1):
                acc = 0
                for b in range(NB):
                    for q in range(NQ):
                        xoff[s][b][q] = acc
                        acc += nidx[s][b][q] // 16

            # ---------------- Phase A: r0 = relu(x @ Wemb + b) ---------------
            with tc.tile_pool(name="psA", bufs=2, space="PSUM") as psA:
                for w in range(NW):
                    sl = slice(w * 128, (w + 1) * 128)
                    xw = iop.tile([128, KT, 128], BF16, tag="xw")
                    nc.sync.dma_start(
                        xw[:],
                        xpre.ap().rearrange("p (w k n) -> p w k n", w=NW, k=KT)[:, w],
                    )
                    ps = psA.tile([128, H], F32, tag="r0")
                    for k in range(KT):
                        nc.tensor.matmul(
                            ps[:], xw[:, k, :], wembP_sb[:, k, :],
                            start=(k == 0), stop=False,
                        )
                    nc.tensor.matmul(
                        ps[:], ones1_sb[:], bemb_sb[:], start=False, stop=True
                    )
                    rkf = stp.tile([128, H], F32, tag="rkf")
                    nc.scalar.activation(rkf[:], ps[:], AF.Relu)
                    nc.sync.dma_start(rloc[0].ap()[sl, :], rkf[:])
                    rk_sb = stp.tile([128, H], BF16, tag="rk")
                    nc.scalar.activation(rk_sb[:], ps[:], AF.Relu)
                    pst = psA.tile([H, 128], BF16, tag="tr")
                    nc.tensor.transpose(pst[:], rk_sb[:], ident[:])
                    nc.scalar.activation(rT0[:H, sl], pst[:], AF.Copy)

            nc.gpsimd.collective_compute(
                "AllGather", mybir.AluOpType.bypass, replica_groups=replica,
                ins=[rloc[0].ap().opt()], outs=[tab[0].ap().opt()],
            )

            # ---------------- Layers ----------------
            for li in (0, 1):
                with tc.tile_pool(name=f"psB{li}", bufs=1, space="PSUM") as psB:
                    for b in range(NB):
                        wlo = b * W_B
                        accbanks = []
                        for k in range((W_B * 2 + 7) // 8):
                            abk = psB.tile([128, 512], F32, tag=f"acc{k}")
                            accbanks.append(abk)

                        def acc_ap(wib, s):
                            i = wib * 2 + s
                            return accbanks[i // 8][:, (i % 8) * 64 : (i % 8) * 64 + 64]
                        # gather + convert all 8 calls of this block
                        msgbf = {}
                        for s in (0, 1):
                            for q in range(NQ):
                                n = nidx[s][b][q]
                                if n == 0:
                                    continue
                                J = n // 128
                                x0 = xoff[s][b][q]
                                it = mep.tile(
                                    [128, max(n // 16, 1)], I16, tag=f"it{s}{q % 2}"
                                )
                                nc.sync.dma_start(
                                    it[:], idx_d[s].ap()[:, x0 : x0 + n // 16]
                                )
                                mtf = mp.tile([128, J, H], F32, tag="mtf")
                                nc.gpsimd.dma_gather(
                                    out_ap=mtf[:],
                                    in_ap=tab[li].ap()[q * QROWS : (q + 1) * QROWS],
                                    idxs_ap=it[:],
                                    num_idxs=n,
                                    num_idxs_reg=n,
                                    elem_size=H,
                                    single_packet=False,
                                )
                                mb_ = mpb.tile([128, J, H], BF16, tag=f"mb{s}{q}")
                                nc.scalar.activation(mb_[:], mtf[:], AF.Copy)
                                msgbf[(s, q)] = mb_
                        # scatter-accumulate pieces into per-window psums
                        ordered = sorted(
                            range(len(sched[b])),
                            key=lambda i: (sched[b][i][3], sched[b][i][0], i),
                        )
                        for i in ordered:
                            s, q, t, wib, col, first, last = sched[b][i]
                            S = mep.tile([128, 128], BF16, tag="S")
                            nc.vector.tensor_tensor(
                                out=S[:],
                                in0=nl_sb[s][:, col : col + 1].to_broadcast(
                                    [128, 128]
                                ),
                                in1=iota_sb[:],
                                op=EQ,
                            )
                            nc.tensor.matmul(
                                acc_ap(wib, s),
                                S[:],
                                msgbf[(s, q)][:, t, :],
                                start=first,
                                stop=last,
                            )
                        # finish windows of this block
                        for wib in range(W_B):
                            w = wlo + wib
                            sl = slice(w * 128, (w + 1) * 128)
                            nT_ps = psB.tile([H, 2, 128], BF16, tag="nT")
                            for s in (0, 1):
                                nsb = stp.tile([128, H], BF16, tag=f"nsb{s}")
                                nc.scalar.activation(
                                    nsb[:], acc_ap(wib, s), AF.Copy,
                                    scale=invd_sb[s][:, w : w + 1],
                                )
                                nc.tensor.transpose(
                                    nT_ps[:, s, :], nsb[:], ident[:]
                                )
                            nT_sb = stp.tile([H, 2, 128], BF16, tag="nTs")
                            nc.scalar.activation(nT_sb[:], nT_ps[:], AF.Copy)
                            rk_ps = psB.tile([128, H], F32, tag="rk")
                            nc.tensor.matmul(
                                rk_ps[:], rT[li][:, sl], wacc_sb[li][:],
                                start=True, stop=False,
                            )
                            nc.tensor.matmul(
                                rk_ps[:], nT_sb[:, 0, :], wb_sb[li][:],
                                start=False, stop=False,
                            )
                            nc.tensor.matmul(
                                rk_ps[:], nT_sb[:, 1, :], wd_sb[li][:],
                                start=False, stop=True,
                            )
                            rk_sb = stp.tile([128, H], BF16, tag="rk")
                            nc.scalar.activation(rk_sb[:], rk_ps[:], AF.Relu)
                            if li == 0:
                                rkf = stp.tile([128, H], F32, tag="rkf")
                                nc.scalar.activation(rkf[:], rk_ps[:], AF.Relu)
                                nc.sync.dma_start(rloc[1].ap()[sl, :], rkf[:])
                            pst = psB.tile([H, 128], BF16, tag="tr")
                            nc.tensor.transpose(pst[:], rk_sb[:], ident[:])
                            nc.scalar.activation(rT[li + 1][:H, sl], pst[:], AF.Copy)
                            if li == 1:
                                lg_ps = psB.tile([128, NCLS], F32, tag="lg")
                                nc.tensor.matmul(
                                    lg_ps[:], rT[0][:, sl], wc0_sb[:],
                                    start=True, stop=False,
                                )
                                nc.tensor.matmul(
                                    lg_ps[:], rT[1][:H, sl], wc1_sb[:],
                                    start=False, stop=False,
                                )
                                nc.tensor.matmul(
                                    lg_ps[:], rT[2][:, sl], wc2_sb[:],
                                    start=False, stop=True,
                                )
                                lg_sb = stp.tile([128, NCLS], F32, tag="lg")
                                nc.scalar.activation(lg_sb[:], lg_ps[:], AF.Copy)
                                nc.sync.dma_start(out_d.ap()[sl, :], lg_sb[:])
                if li == 0:
                    nc.gpsimd.collective_compute(
                        "AllGather", mybir.AluOpType.bypass,
                        replica_groups=replica,
                        ins=[rloc[1].ap().opt()], outs=[tab[1].ap().opt()],
                    )

    nc.compile()
    return nc


def prepare(x, edge_index_1, edge_index_2, W_embed, b_embed, W0, b0, W1, b1, Wc, bc,
            ncores=NCORES):
    x = np.asarray(x, np.float32)
    n_nodes, in_dim = x.shape
    npc = n_nodes // ncores
    NW = (npc + 127) // 128
    NP = NW * 128
    NTAB = ncores * NP
    ipad = ((in_dim + 127) // 128) * 128
    KT = ipad // 128
    ncls = np.asarray(Wc).shape[1]
    NB = NW // W_B
    assert NW % W_B == 0
    QROWS = NTAB // NQ
    assert NTAB % NQ == 0 and QROWS < 32768

    e = [np.asarray(edge_index_1), np.asarray(edge_index_2)]
    src = [np.asarray(ei[0], np.int64) for ei in e]
    dst = [np.asarray(ei[1], np.int64) for ei in e]
    deg = [np.bincount(s, minlength=n_nodes) for s in src]

    # table row of node v: c_v*NP + (v - c_v*npc)
    cv = np.arange(n_nodes) // npc
    tabid_all = cv * NP + (np.arange(n_nodes) - cv * npc)

    # per (set, core): edge (o, t): window w = local(o)//128, partition
    # p = local(o)%128, quarter q = t//QROWS, local row = t%QROWS
    percore = [[None] * ncores for _ in (0, 1)]
    cnt_wq = [np.zeros((ncores, NW, NQ), np.int64) for _ in (0, 1)]
    for s in (0, 1):
        for c in range(ncores):
            lo = c * npc
            m = (src[s] >= lo) & (src[s] < lo + npc)
            ol = src[s][m] - lo
            t = tabid_all[dst[s][m]]
            w = ol // 128
            q = t // QROWS
            order = np.lexsort((t, q, w))
            ol, t, w, q = ol[order], t[order], w[order], q[order]
            percore[s][c] = (ol, t, w, q)
            np.add.at(cnt_wq[s][c], (w, q), 1)

    # cross-core max run lengths M[s][w][q]
    M = [cnt_wq[s].max(axis=0) for s in (0, 1)]  # [NW, NQ]

    # per-call padded num_idxs
    nidx = [[[0] * NQ for _ in range(NB)] for _ in (0, 1)]
    for s in (0, 1):
        for b in range(NB):
            for q in range(NQ):
                tot = int(M[s][b * W_B : (b + 1) * W_B, q].sum())
                tot = max(((tot + 127) // 128) * 128, 128)
                nidx[s][b][q] = tot

    # piece schedules + nl label columns + idx arrays
    sched = [[] for _ in range(NB)]
    npieces = [0, 0]
    piece_meta = [[], []]  # (s): list of (b, q, tile, w_in_b, lo, hi, part_lo)
    for b in range(NB):
        ent = []
        for s in (0, 1):
            for q in range(NQ):
                cur = 0
                pieces_of_w = {}
                for wib in range(W_B):
                    w = b * W_B + wib
                    run = int(M[s][w, q])
                    a, z = cur, cur + run
                    cur = z
                    while a < z:
                        t = a // 128
                        hi = min(z, (t + 1) * 128)
                        pieces_of_w.setdefault((wib, s), []).append(
                            (s, q, t, wib, a - t * 128, hi - t * 128)
                        )
                        a = hi
                for key, plist in pieces_of_w.items():
                    for pp in plist:
                        ent.append(pp)
        # mark first/last per (wib, s) across the whole block
        seen = {}
        for i, (s, q, t, wib, plo, phi) in enumerate(ent):
            seen.setdefault((wib, s), []).append(i)
        final = [None] * len(ent)
        for key, idxs in seen.items():
            for j, i in enumerate(idxs):
                s, q, t, wib, plo, phi = ent[i]
                col = npieces[s]
                npieces[s] += 1
                piece_meta[s].append((b, q, t, wib, plo, phi, col))
                final[i] = (s, q, t, wib, col, j == 0, j == len(idxs) - 1)
        sched[b] = final

    # host-side per-core arrays
    XW = [sum(nidx[s][b][q] for b in range(NB) for q in range(NQ)) // 16
          for s in (0, 1)]
    idx_arrs = [
        [np.zeros((128, XW[s]), np.int16) for _ in range(ncores)] for s in (0, 1)
    ]
    nl_arrs = [
        [np.full((128, npieces[s]), -1, np.float32) for _ in range(ncores)]
        for s in (0, 1)
    ]
    for s in (0, 1):
        for c in range(ncores):
            ol, t, w, q = percore[s][c]
            # call-local positions: slots of (w, q) run start at cumulative M
            xo = 0
            wrapped = idx_arrs[s][c]
            for b in range(NB):
                for qq in range(NQ):
                    n = nidx[s][b][qq]
                    base = np.zeros(n, np.int64)  # pad idx -> row 0 of quarter
                    labels = {}
                    cur = 0
                    for wib in range(W_B):
                        ww = b * W_B + wib
                        mm = (w == ww) & (q == qq)
                        k = int(mm.sum())
                        base[cur : cur + k] = t[mm] % QROWS
                        lab = ol[mm] % 128
                        for tt in range(cur // 128, ((cur + k - 1) // 128) + 1 if k else cur // 128):
                            pass
                        labels[wib] = (cur, k, lab)
                        cur += int(M[s][ww, qq])
                    # wrapped int16 layout, replicated to 8 stripes
                    wr = base.reshape(n // 16, 16).T.astype(np.int16)
                    for st in range(8):
                        wrapped[st * 16 : (st + 1) * 16, xo : xo + n // 16] = wr
                    xo += n // 16
                    # nl labels for this call's pieces
                    for (bb, qq2, tt, wib, plo, phi, col) in piece_meta[s]:
                        if bb != b or qq2 != qq:
                            continue
                        cur0, k, lab = labels[wib]
                        # piece covers slots [tt*128+plo, tt*128+phi) of the call
                        a = tt * 128 + plo
                        z = tt * 128 + phi
                        # valid slots of this window's run in [cur0, cur0+k)
                        va = max(a, cur0)
                        vz = min(z, cur0 + k)
                        if vz > va:
                            nl_arrs[s][c][va - tt * 128 : vz - tt * 128, col] = lab[
                                va - cur0 : vz - cur0
                            ]

    def b16(a):
        return np.asarray(a, np.float32).astype(bfloat16)

    W_embed = np.asarray(W_embed, np.float32)
    wembP = np.zeros((128, KT, H), np.float32)
    for k in range(KT):
        rows = W_embed[k * 128 : min((k + 1) * 128, in_dim)]
        wembP[: rows.shape[0], k, :] = rows
    wembP = b16(wembP.reshape(128, KT * H))

    def layer_w(W, b):
        W = np.asarray(W, np.float32)
        acc = np.zeros((H + 1, H), np.float32)
        acc[:H] = W[0:H] + W[2 * H : 3 * H]
        acc[H] = np.asarray(b, np.float32)
        return b16(acc), b16(W[H : 2 * H]), b16(W[3 * H : 4 * H])

    wacc0, wb0, wd0 = layer_w(W0, b0)
    wacc1, wb1, wd1 = layer_w(W1, b1)
    Wc = np.asarray(Wc, np.float32)
    wc0 = np.zeros((H + 1, ncls), np.float32)
    wc0[:H] = Wc[0:H]
    wc0[H] = np.asarray(bc, np.float32)

    shared = {
        "wembP": wembP,
        "brow_emb": b16(np.asarray(b_embed, np.float32).reshape(1, H)),
        "wacc0": wacc0, "wb0": wb0, "wd0": wd0,
        "wacc1": wacc1, "wb1": wb1, "wd1": wd1,
        "wc0": b16(wc0),
        "wc1": b16(Wc[H : 2 * H]),
        "wc2": b16(Wc[2 * H : 3 * H]),
        "onesrow": np.ones((1, NP), bfloat16),
        "iota": np.broadcast_to(
            np.arange(128, dtype=np.float32), (128, 128)
        ).astype(bfloat16),
    }

    in_maps = []
    xb = x.astype(bfloat16)
    for c in range(ncores):
        lo = c * npc
        Xl = np.zeros((NP, ipad), bfloat16)
        Xl[:npc, :in_dim] = xb[lo : lo + npc]
        xpre = np.ascontiguousarray(
            Xl.reshape(NW, 128, KT, 128).transpose(3, 0, 2, 1)
        ).reshape(128, NW * KT * 128)
        m = dict(shared)
        m["xpre"] = xpre
        for s in (0, 1):
            m[f"idx{s}"] = idx_arrs[s][c]
            m[f"nl{s}"] = nl_arrs[s][c].astype(bfloat16)
            iv = (1.0 / np.maximum(deg[s][lo : lo + npc], 1)).astype(np.float32)
            iv = np.concatenate([iv, np.ones(NP - npc, np.float32)])
            m[f"invd{s}"] = np.ascontiguousarray(iv.reshape(NW, 128).T)
        in_maps.append(m)

    cfg = dict(
        NW=NW, KT=KT, NCLS=ncls, NP=NP, NTAB=NTAB,
        nidx=nidx, sched=sched, npieces=npieces,
        n_nodes=n_nodes, npc=npc, ncores=ncores,
    )
    return cfg, in_maps


def unshard(cfg, results):
    n_nodes, npc = cfg["n_nodes"], cfg["npc"]
    out = np.empty((n_nodes, cfg["NCLS"]), np.float32)
    for c in range(cfg["ncores"]):
        out[c * npc : (c + 1) * npc] = results[c]["logits"][:npc]
    return out


_CACHE = {}


def kernel(**inputs):
    cfg, in_maps = prepare(**inputs)
    key = str(cfg["nidx"]) + str(cfg["npieces"])
    if key not in _CACHE:
        _CACHE[key] = build_program(cfg)
    nc = _CACHE[key]
    res = run_bass_kernel_spmd(nc, in_maps, core_ids=list(range(cfg["ncores"])))
    return unshard(cfg, res.results)
